# revision 21
# baseline (speedup 1.0000x reference)
"""Segment-mean pooling kernel for Trainium2 (8 NeuronCores, data-parallel).

Input : emb_vector [1024, 2048, 64] f32
Output: [1024, 32, 64] f32 — mean over 32 ragged field segments
        (sizes [32, 64, 96, 64] * 8, summing to 2048).

Sharding: batch axis 0 split across 8 cores (128 rows each). Per core the
128 batch rows sit on the 128 SBUF partitions; fields*embed is the free
axis. The segment pattern repeats every 256 fields, so each core streams 8
groups of [128, 256*64] f32 (64 KiB/partition, contiguous in DRAM) and
reduces each group's 4 segments on the vector engine with a strided-X
reduce, then scales by 1/size and DMAs the [128, 4, 64] result out.
"""

import os
import sys
from functools import lru_cache

import numpy as np

for _p in ("/opt/trn_rl_repo", os.path.expanduser("~/.axon_site/_ro/trn_rl_repo")):
    if os.path.isdir(_p) and _p not in sys.path:
        sys.path.insert(0, _p)

import concourse.bass as bass
import concourse.bacc as bacc
import concourse.mybir as mybir
from concourse import tile

N_CORES = 8
BATCH, FIELDS, D = 1024, 2048, 64
B_LOC = BATCH // N_CORES          # 128 batch rows per core = SBUF partitions
GROUP_F = 256                     # fields per repeating segment group
GROUPS = FIELDS // GROUP_F        # 8
SEG_OFF = (0, 32, 96, 192)        # field offsets within a group
SEG_SZ = (32, 64, 96, 64)         # segment sizes
NSEG_G = 4                        # segments per group
NSEG = NSEG_G * GROUPS            # 32
FP32 = mybir.dt.float32


def _emit_group(nc, t, o, variant: str):
    """Reduce one group tile t [128, 256*64] into segment means o [128, 4*64].

    variant 'strided': 4 strided-X vector reduces (v1).
    variant 'tree': in-place contiguous pairwise fold — every segment is a
    multiple of 32 fields, so fold each 32-field block down to one 64-wide
    block sum (contiguous TT adds run at 1 elem/cycle vs ~1.5 for strided
    reduce), then combine blocks per segment with small strided reduces.
    """
    BLK = 32 * D  # one folded 32-field block: 2048 elems
    if variant == "strided":
        t3 = t[:].rearrange("b (f d) -> b d f", d=D)
        for si in range(NSEG_G):
            f0, sz = SEG_OFF[si], SEG_SZ[si]
            nc.vector.reduce_sum(
                out=o[:, si * D : (si + 1) * D],
                in_=t3[:, :, f0 : f0 + sz],
                axis=mybir.AxisListType.X,
            )
            nc.scalar.mul(
                out=o[:, si * D : (si + 1) * D],
                in_=o[:, si * D : (si + 1) * D],
                mul=1.0 / sz,
            )
        return

    # tree: view [b, blk, within]: fold `within` 1024->512->...->64 in place
    for width in (1024, 512, 256, 128, 64):
        lhs = t[:].rearrange("b (k w) -> b k w", w=BLK)[:, :, :width]
        rhs = t[:].rearrange("b (k w) -> b k w", w=BLK)[:, :, width : 2 * width]
        nc.vector.tensor_add(lhs, lhs, rhs)
    # block sums now at t[:, k*BLK : k*BLK + 64] for k in 0..7
    blocks = t[:].rearrange("b (k w) -> b w k", w=BLK)[:, :D, :]  # [b, d, k]
    seg_blocks = ((0, 1), (1, 3), (3, 6), (6, 8))
    for si, (k0, k1) in enumerate(seg_blocks):
        osl = o[:, si * D : (si + 1) * D]
        if k1 - k0 == 1:
            nc.scalar.activation(
                out=osl,
                in_=t[:, k0 * BLK : k0 * BLK + D],
                func=mybir.ActivationFunctionType.Copy,
                scale=1.0 / SEG_SZ[si],
            )
        else:
            nc.vector.reduce_sum(
                out=osl, in_=blocks[:, :, k0:k1], axis=mybir.AxisListType.X
            )
            nc.scalar.mul(out=osl, in_=osl, mul=1.0 / SEG_SZ[si])


@lru_cache(maxsize=8)
def _build(reps: int = 1, variant: str = "tree"):
    """reps>1 repeats the whole workload back-to-back inside one NEFF —
    used only for timing (marginal per-rep time cancels dispatch+preamble
    overheads)."""
    nc = bacc.Bacc(
        "TRN2", target_bir_lowering=False, debug=False, num_devices=N_CORES
    )
    x = nc.declare_dram_parameter("x", [B_LOC, FIELDS, D], FP32, isOutput=False)
    y = nc.declare_dram_parameter("y", [B_LOC, NSEG, D], FP32, isOutput=True)
    xf = x.rearrange("b f d -> b (f d)")

    with tile.TileContext(nc) as tc:
        with (
            tc.tile_pool(name="inp", bufs=2) as inp_pool,
            tc.tile_pool(name="outp", bufs=2) as out_pool,
        ):
            for _ in range(reps):
                for g in range(GROUPS):
                    t = inp_pool.tile([B_LOC, GROUP_F * D], FP32, tag="in")
                    nc.sync.dma_start(
                        out=t[:],
                        in_=xf[:, g * GROUP_F * D : (g + 1) * GROUP_F * D],
                    )
                    o = out_pool.tile([B_LOC, NSEG_G * D], FP32, tag="out")
                    _emit_group(nc, t, o, variant)
                    nc.sync.dma_start(
                        out=y[:, g * NSEG_G : (g + 1) * NSEG_G, :],
                        in_=o[:].rearrange("b (s d) -> b s d", d=D),
                    )
    nc.finalize()
    return nc


def _sharded_from_nc(nc):
    """Build the 8-way-sharded jitted executable for a finalized Bass module.

    Mirrors bass2jax.run_bass_via_pjrt's multi-core branch (shard_map over a
    'core' mesh; per-device shard == the BIR-declared per-core shape) but
    without output-buffer donation so the same function can be called in a
    timing loop with device-resident inputs.
    """
    import jax
    from jax.experimental.shard_map import shard_map
    from jax.sharding import Mesh, NamedSharding, PartitionSpec

    from concourse import bass2jax, mybir as _mybir

    bass2jax.install_neuronx_cc_hook()

    in_names, out_names, out_avals, zero_outs = [], [], [], []
    partition_name = (
        nc.partition_id_tensor.name if nc.partition_id_tensor else None
    )
    for alloc in nc.m.functions[0].allocations:
        if not isinstance(alloc, _mybir.MemoryLocationSet):
            continue
        name = alloc.memorylocations[0].name
        if alloc.kind == "ExternalInput":
            if name != partition_name:
                in_names.append(name)
        elif alloc.kind == "ExternalOutput":
            shape = tuple(alloc.tensor_shape)
            dtype = _mybir.dt.np(alloc.dtype)
            out_names.append(name)
            out_avals.append(jax.core.ShapedArray(shape, dtype))
            zero_outs.append(np.zeros(shape, dtype))
    n_params = len(in_names)
    all_in_names = list(in_names) + list(out_names)
    if partition_name is not None:
        all_in_names.append(partition_name)

    def _body(*args):
        operands = list(args)
        if partition_name is not None:
            operands.append(bass2jax.partition_id_tensor())
        outs = bass2jax._bass_exec_p.bind(
            *operands,
            out_avals=tuple(out_avals),
            in_names=tuple(all_in_names),
            out_names=tuple(out_names),
            lowering_input_output_aliases=(),
            sim_require_finite=True,
            sim_require_nnan=True,
            nc=nc,
        )
        return tuple(outs)

    devices = jax.devices()[:N_CORES]
    mesh = Mesh(np.asarray(devices), ("core",))
    n_outs = len(out_names)
    in_specs = (PartitionSpec("core"),) * (n_params + n_outs)
    out_specs = (PartitionSpec("core"),) * n_outs
    sharded = jax.jit(
        shard_map(
            _body, mesh=mesh, in_specs=in_specs, out_specs=out_specs,
            check_rep=False,
        ),
        keep_unused=True,
    )
    in_sharding = NamedSharding(mesh, PartitionSpec("core"))
    return sharded, zero_outs, in_sharding


@lru_cache(maxsize=4)
def _compiled(reps: int = 1):
    return _sharded_from_nc(_build(reps))


def _put_inputs(emb_vector: np.ndarray, reps: int = 1):
    import jax

    sharded, zero_outs, in_sharding = _compiled(reps)
    x = np.ascontiguousarray(emb_vector, dtype=np.float32)
    dx = jax.device_put(x, in_sharding)
    dzeros = [
        jax.device_put(
            np.zeros((N_CORES * z.shape[0], *z.shape[1:]), z.dtype), in_sharding
        )
        for z in zero_outs
    ]
    return sharded, dx, dzeros


def kernel(emb_vector: np.ndarray) -> np.ndarray:
    sharded, dx, dzeros = _put_inputs(emb_vector)
    (out,) = sharded(dx, *dzeros)
    return np.asarray(out)


def bench(emb_vector: np.ndarray, iters: int = 30, warmup: int = 5,
          reps: int = 1):
    """Steady-state per-call wall time of the sharded executable, ns."""
    import time

    sharded, dx, dzeros = _put_inputs(emb_vector, reps)
    for _ in range(warmup):
        (out,) = sharded(dx, *dzeros)
    out.block_until_ready()
    t0 = time.perf_counter()
    for _ in range(iters):
        (out,) = sharded(dx, *dzeros)
    out.block_until_ready()
    t1 = time.perf_counter()
    return (t1 - t0) / iters * 1e9, np.asarray(out)


def measure_exec_ns(emb_vector: np.ndarray, lo: int = 4, hi: int = 12,
                    iters: int = 20, warmup: int = 5):
    """Marginal per-execution HW time via in-NEFF workload repetition:
    (t(hi reps) - t(lo reps)) / (hi - lo) cancels per-dispatch client/RPC
    overhead and NEFF preamble/postamble."""
    t_hi, out = bench(emb_vector, iters=iters, warmup=warmup, reps=hi)
    t_lo, _ = bench(emb_vector, iters=iters, warmup=warmup, reps=lo)
    return (t_hi - t_lo) / (hi - lo), out


# revision 26
# speedup vs baseline: 1.0570x; 1.0570x over previous
"""Segment-mean pooling kernel for Trainium2 (8 NeuronCores, data-parallel).

Input : emb_vector [1024, 2048, 64] f32
Output: [1024, 32, 64] f32 — mean over 32 ragged field segments
        (sizes [32, 64, 96, 64] * 8, summing to 2048).

Sharding: batch axis 0 split across 8 cores (128 rows each). Per core the
128 batch rows sit on the 128 SBUF partitions; fields*embed is the free
axis. The segment pattern repeats every 256 fields, so each core streams 8
groups of [128, 256*64] f32 (64 KiB/partition, contiguous in DRAM) and
reduces each group's 4 segments on the vector engine with a strided-X
reduce, then scales by 1/size and DMAs the [128, 4, 64] result out.
"""

import os
import sys
from functools import lru_cache

import numpy as np

for _p in ("/opt/trn_rl_repo", os.path.expanduser("~/.axon_site/_ro/trn_rl_repo")):
    if os.path.isdir(_p) and _p not in sys.path:
        sys.path.insert(0, _p)

import concourse.bass as bass
import concourse.bacc as bacc
import concourse.mybir as mybir
from concourse import tile

N_CORES = 8
BATCH, FIELDS, D = 1024, 2048, 64
B_LOC = BATCH // N_CORES          # 128 batch rows per core = SBUF partitions
GROUP_F = 256                     # fields per repeating segment group
GROUPS = FIELDS // GROUP_F        # 8
SEG_OFF = (0, 32, 96, 192)        # field offsets within a group
SEG_SZ = (32, 64, 96, 64)         # segment sizes
NSEG_G = 4                        # segments per group
NSEG = NSEG_G * GROUPS            # 32
FP32 = mybir.dt.float32


def _emit_group(nc, t, o, variant: str):
    """Reduce one group tile t [128, 256*64] into segment means o [128, 4*64].

    variant 'strided': 4 strided-X vector reduces (v1).
    variant 'tree': in-place contiguous pairwise fold — every segment is a
    multiple of 32 fields, so fold each 32-field block down to one 64-wide
    block sum (contiguous TT adds run at 1 elem/cycle vs ~1.5 for strided
    reduce), then combine blocks per segment with small strided reduces.
    """
    BLK = 32 * D  # one folded 32-field block: 2048 elems
    if variant == "strided":
        t3 = t[:].rearrange("b (f d) -> b d f", d=D)
        for si in range(NSEG_G):
            f0, sz = SEG_OFF[si], SEG_SZ[si]
            nc.vector.reduce_sum(
                out=o[:, si * D : (si + 1) * D],
                in_=t3[:, :, f0 : f0 + sz],
                axis=mybir.AxisListType.X,
            )
            nc.scalar.mul(
                out=o[:, si * D : (si + 1) * D],
                in_=o[:, si * D : (si + 1) * D],
                mul=1.0 / sz,
            )
        return

    if variant == "tree":
        # view [b, blk, within]: fold `within` 1024->512->...->64 in place
        for width in (1024, 512, 256, 128, 64):
            lhs = t[:].rearrange("b (k w) -> b k w", w=BLK)[:, :, :width]
            rhs = t[:].rearrange("b (k w) -> b k w", w=BLK)[
                :, :, width : 2 * width
            ]
            nc.vector.tensor_add(lhs, lhs, rhs)
        # block sums now at t[:, k*BLK : k*BLK + 64] for k in 0..7
        blocks = t[:].rearrange("b (k w) -> b w k", w=BLK)[:, :D, :]
        seg_blocks = ((0, 1), (1, 3), (3, 6), (6, 8))
        for si, (k0, k1) in enumerate(seg_blocks):
            osl = o[:, si * D : (si + 1) * D]
            if k1 - k0 == 1:
                nc.scalar.activation(
                    out=osl,
                    in_=t[:, k0 * BLK : k0 * BLK + D],
                    func=mybir.ActivationFunctionType.Copy,
                    scale=1.0 / SEG_SZ[si],
                )
            else:
                nc.vector.reduce_sum(
                    out=osl, in_=blocks[:, :, k0:k1], axis=mybir.AxisListType.X
                )
                nc.scalar.mul(out=osl, in_=osl, mul=1.0 / SEG_SZ[si])
        return

    assert variant == "hybrid"
    # One contiguous in-place fold level (each 32-field block: fields
    # [0:16) += [16:32)), then one strided XY-reduce per segment over the
    # folded fields of its blocks.
    v = t[:].rearrange("b (k w) -> b k w", w=BLK)
    nc.vector.tensor_add(v[:, :, :1024], v[:, :, :1024], v[:, :, 1024:2048])
    # folded tile view [b, k, f(16), d] -> reduce per segment over (k, f)
    t4 = t[:].rearrange("b (k f d) -> b d k f", k=8, d=D)  # [b, d, k, f16]
    seg_blocks = ((0, 1), (1, 3), (3, 6), (6, 8))
    for si, (k0, k1) in enumerate(seg_blocks):
        osl = o[:, si * D : (si + 1) * D]
        nc.vector.reduce_sum(
            out=osl,
            in_=t4[:, :, k0:k1, :16],
            axis=mybir.AxisListType.XY,
        )
        nc.scalar.mul(out=osl, in_=osl, mul=1.0 / SEG_SZ[si])


@lru_cache(maxsize=16)
def _build(reps: int = 1, variant: str = "tree", chunk_f: int = 256,
           bufs: int = 2, out_eng: str = "scalar"):
    """reps>1 repeats the whole workload back-to-back inside one NEFF —
    used only for timing (marginal per-rep time cancels dispatch+preamble
    overheads)."""
    nc = bacc.Bacc(
        "TRN2", target_bir_lowering=False, debug=False, num_devices=N_CORES
    )
    x = nc.declare_dram_parameter("x", [B_LOC, FIELDS, D], FP32, isOutput=False)
    y = nc.declare_dram_parameter("y", [B_LOC, NSEG, D], FP32, isOutput=True)
    xf = x.rearrange("b f d -> b (f d)")

    with tile.TileContext(nc) as tc:
        with (
            tc.tile_pool(name="inp", bufs=bufs) as inp_pool,
            tc.tile_pool(name="outp", bufs=2) as out_pool,
            tc.tile_pool(name="tmpp", bufs=2) as tmp_pool,
        ):
            for _ in range(reps):
                if chunk_f == GROUP_F:
                    o_all = None
                    if out_eng == "final":
                        o_all = out_pool.tile([B_LOC, NSEG * D], FP32,
                                              tag="oall")
                    for g in range(GROUPS):
                        t = inp_pool.tile(
                            [B_LOC, GROUP_F * D], FP32, tag="in"
                        )
                        nc.sync.dma_start(
                            out=t[:],
                            in_=xf[:, g * GROUP_F * D : (g + 1) * GROUP_F * D],
                        )
                        if out_eng == "final":
                            o = o_all[:, g * NSEG_G * D : (g + 1) * NSEG_G * D]
                            _emit_group(nc, t, o, variant)
                        else:
                            o = out_pool.tile([B_LOC, NSEG_G * D], FP32,
                                              tag="out")
                            _emit_group(nc, t, o[:], variant)
                            dma_eng = {
                                "sync": nc.sync,
                                "gpsimd": nc.gpsimd,
                                "scalar": nc.scalar,
                            }[out_eng]
                            dma_eng.dma_start(
                                out=y[:, g * NSEG_G : (g + 1) * NSEG_G, :],
                                in_=o[:].rearrange("b (s d) -> b s d", d=D),
                            )
                    if out_eng == "final":
                        nc.scalar.dma_start(
                            out=y[:, :, :],
                            in_=o_all[:].rearrange("b (s d) -> b s d", d=D),
                        )
                else:
                    assert chunk_f == GROUP_F // 2 and variant == "strided"
                    HF = chunk_f * D  # 8192
                    for g in range(GROUPS):
                        o = out_pool.tile([B_LOC, NSEG_G * D], FP32, tag="out")
                        for h in range(2):
                            t = inp_pool.tile([B_LOC, HF], FP32, tag="in")
                            nc.sync.dma_start(
                                out=t[:],
                                in_=xf[
                                    :,
                                    (2 * g + h) * HF : (2 * g + h + 1) * HF,
                                ],
                            )
                            t3 = t[:].rearrange("b (f d) -> b d f", d=D)
                            if h == 0:
                                # fields 0:128 = seg0(32), seg1(64), seg2a(32)
                                nc.vector.reduce_sum(
                                    out=o[:, 0:D], in_=t3[:, :, 0:32],
                                    axis=mybir.AxisListType.X,
                                )
                                nc.vector.reduce_sum(
                                    out=o[:, D : 2 * D], in_=t3[:, :, 32:96],
                                    axis=mybir.AxisListType.X,
                                )
                                nc.vector.reduce_sum(
                                    out=o[:, 2 * D : 3 * D],
                                    in_=t3[:, :, 96:128],
                                    axis=mybir.AxisListType.X,
                                )
                            else:
                                # fields 128:256 = seg2b(64), seg3(64)
                                tmp = tmp_pool.tile([B_LOC, D], FP32, tag="t2")
                                nc.vector.reduce_sum(
                                    out=tmp[:], in_=t3[:, :, 0:64],
                                    axis=mybir.AxisListType.X,
                                )
                                nc.vector.tensor_add(
                                    o[:, 2 * D : 3 * D], o[:, 2 * D : 3 * D],
                                    tmp[:],
                                )
                                nc.vector.reduce_sum(
                                    out=o[:, 3 * D : 4 * D],
                                    in_=t3[:, :, 64:128],
                                    axis=mybir.AxisListType.X,
                                )
                        for si in range(NSEG_G):
                            nc.scalar.mul(
                                out=o[:, si * D : (si + 1) * D],
                                in_=o[:, si * D : (si + 1) * D],
                                mul=1.0 / SEG_SZ[si],
                            )
                        dma_eng = nc.sync if out_eng == "sync" else nc.gpsimd
                        dma_eng.dma_start(
                            out=y[:, g * NSEG_G : (g + 1) * NSEG_G, :],
                            in_=o[:].rearrange("b (s d) -> b s d", d=D),
                        )
    nc.finalize()
    return nc


def _sharded_from_nc(nc):
    """Build the 8-way-sharded jitted executable for a finalized Bass module.

    Mirrors bass2jax.run_bass_via_pjrt's multi-core branch (shard_map over a
    'core' mesh; per-device shard == the BIR-declared per-core shape) but
    without output-buffer donation so the same function can be called in a
    timing loop with device-resident inputs.
    """
    import jax
    from jax.experimental.shard_map import shard_map
    from jax.sharding import Mesh, NamedSharding, PartitionSpec

    from concourse import bass2jax, mybir as _mybir

    bass2jax.install_neuronx_cc_hook()

    in_names, out_names, out_avals, zero_outs = [], [], [], []
    partition_name = (
        nc.partition_id_tensor.name if nc.partition_id_tensor else None
    )
    for alloc in nc.m.functions[0].allocations:
        if not isinstance(alloc, _mybir.MemoryLocationSet):
            continue
        name = alloc.memorylocations[0].name
        if alloc.kind == "ExternalInput":
            if name != partition_name:
                in_names.append(name)
        elif alloc.kind == "ExternalOutput":
            shape = tuple(alloc.tensor_shape)
            dtype = _mybir.dt.np(alloc.dtype)
            out_names.append(name)
            out_avals.append(jax.core.ShapedArray(shape, dtype))
            zero_outs.append(np.zeros(shape, dtype))
    n_params = len(in_names)
    all_in_names = list(in_names) + list(out_names)
    if partition_name is not None:
        all_in_names.append(partition_name)

    def _body(*args):
        operands = list(args)
        if partition_name is not None:
            operands.append(bass2jax.partition_id_tensor())
        outs = bass2jax._bass_exec_p.bind(
            *operands,
            out_avals=tuple(out_avals),
            in_names=tuple(all_in_names),
            out_names=tuple(out_names),
            lowering_input_output_aliases=(),
            sim_require_finite=True,
            sim_require_nnan=True,
            nc=nc,
        )
        return tuple(outs)

    devices = jax.devices()[:N_CORES]
    mesh = Mesh(np.asarray(devices), ("core",))
    n_outs = len(out_names)
    in_specs = (PartitionSpec("core"),) * (n_params + n_outs)
    out_specs = (PartitionSpec("core"),) * n_outs
    sharded = jax.jit(
        shard_map(
            _body, mesh=mesh, in_specs=in_specs, out_specs=out_specs,
            check_rep=False,
        ),
        keep_unused=True,
    )
    in_sharding = NamedSharding(mesh, PartitionSpec("core"))
    return sharded, zero_outs, in_sharding


@lru_cache(maxsize=4)
def _compiled(reps: int = 1):
    return _sharded_from_nc(_build(reps))


def _put_inputs(emb_vector: np.ndarray, reps: int = 1):
    import jax

    sharded, zero_outs, in_sharding = _compiled(reps)
    x = np.ascontiguousarray(emb_vector, dtype=np.float32)
    dx = jax.device_put(x, in_sharding)
    dzeros = [
        jax.device_put(
            np.zeros((N_CORES * z.shape[0], *z.shape[1:]), z.dtype), in_sharding
        )
        for z in zero_outs
    ]
    return sharded, dx, dzeros


def kernel(emb_vector: np.ndarray) -> np.ndarray:
    sharded, dx, dzeros = _put_inputs(emb_vector)
    (out,) = sharded(dx, *dzeros)
    return np.asarray(out)


def bench(emb_vector: np.ndarray, iters: int = 30, warmup: int = 5,
          reps: int = 1):
    """Steady-state per-call wall time of the sharded executable, ns."""
    import time

    sharded, dx, dzeros = _put_inputs(emb_vector, reps)
    for _ in range(warmup):
        (out,) = sharded(dx, *dzeros)
    out.block_until_ready()
    t0 = time.perf_counter()
    for _ in range(iters):
        (out,) = sharded(dx, *dzeros)
    out.block_until_ready()
    t1 = time.perf_counter()
    return (t1 - t0) / iters * 1e9, np.asarray(out)


def measure_exec_ns(emb_vector: np.ndarray, lo: int = 4, hi: int = 12,
                    iters: int = 20, n_pairs: int = 7):
    """Marginal per-execution HW time via in-NEFF workload repetition:
    (t(hi reps) - t(lo reps)) / (hi - lo) cancels per-dispatch client/RPC
    overhead and NEFF preamble/postamble. hi/lo timing loops are
    interleaved (median of per-pair diffs) so device-load drift cancels."""
    import time

    sharded_hi, dx, dz_hi = _put_inputs(emb_vector, hi)
    sharded_lo, _, dz_lo = _put_inputs(emb_vector, lo)
    for _ in range(4):
        (out,) = sharded_hi(dx, *dz_hi)
        (out_lo,) = sharded_lo(dx, *dz_lo)
    out.block_until_ready()
    out_lo.block_until_ready()
    diffs = []
    for _ in range(n_pairs):
        t0 = time.perf_counter()
        for _ in range(iters):
            (out,) = sharded_hi(dx, *dz_hi)
        out.block_until_ready()
        t1 = time.perf_counter()
        for _ in range(iters):
            (out_lo,) = sharded_lo(dx, *dz_lo)
        out_lo.block_until_ready()
        t2 = time.perf_counter()
        diffs.append(((t1 - t0) - (t2 - t1)) / iters * 1e9)
    med = sorted(diffs)[len(diffs) // 2]
    return med / (hi - lo), np.asarray(out)


# revision 30
# speedup vs baseline: 1.1864x; 1.1224x over previous
"""Segment-mean pooling kernel for Trainium2 (8 NeuronCores, data-parallel).

Input : emb_vector [1024, 2048, 64] f32
Output: [1024, 32, 64] f32 — mean over 32 ragged field segments
        (sizes [32, 64, 96, 64] * 8, summing to 2048).

Sharding: batch axis 0 split across 8 cores (128 rows each). Per core the
128 batch rows sit on the 128 SBUF partitions; fields*embed is the free
axis. The segment pattern repeats every 256 fields, so each core streams 8
groups of [128, 256*64] f32 (64 KiB/partition, contiguous in DRAM; 8 MiB
per DMA, double-buffered -> DMA runs at the ~358 GB/s HBM-per-core limit).

Per group the reduction runs as an in-place contiguous pairwise fold: every
segment is a whole number of 32-field blocks, so each block folds
1024->512->256->128->64 elems with stride-1 tensor_adds (1 elem/cycle on
DVE; a strided-X reduce measures ~1.5 cycles/elem on TRN2), then block sums
combine per segment. Segment 3's two blocks fold on GPSIMD instead of DVE
(measured -9% end to end: pool has its own SBUF path, cutting DVE<->DMA
port contention). Scale-by-1/size and the output DMA issue from the ACT
engine so the SP sequencer's HWDGE ring only ever streams input loads
(out-DMA sem-waits on SP bubble the input stream; measured ~+18 us).

Measured marginal per-execution time: ~210-230 us on a quiet device vs a
~188 us pure-DMA floor (65 MiB/core at the HBM limit); device-sharing
bursts inflate both.
"""

import os
import sys
from functools import lru_cache

import numpy as np

for _p in ("/opt/trn_rl_repo", os.path.expanduser("~/.axon_site/_ro/trn_rl_repo")):
    if os.path.isdir(_p) and _p not in sys.path:
        sys.path.insert(0, _p)

import concourse.bass as bass
import concourse.bacc as bacc
import concourse.mybir as mybir
from concourse import tile

N_CORES = 8
BATCH, FIELDS, D = 1024, 2048, 64
B_LOC = BATCH // N_CORES          # 128 batch rows per core = SBUF partitions
GROUP_F = 256                     # fields per repeating segment group
GROUPS = FIELDS // GROUP_F        # 8
SEG_OFF = (0, 32, 96, 192)        # field offsets within a group
SEG_SZ = (32, 64, 96, 64)         # segment sizes
NSEG_G = 4                        # segments per group
NSEG = NSEG_G * GROUPS            # 32
FP32 = mybir.dt.float32


def _emit_group(nc, t, o, variant: str):
    """Reduce one group tile t [128, 256*64] into segment means o [128, 4*64].

    variant 'strided': 4 strided-X vector reduces (v1).
    variant 'tree': in-place contiguous pairwise fold — every segment is a
    multiple of 32 fields, so fold each 32-field block down to one 64-wide
    block sum (contiguous TT adds run at 1 elem/cycle vs ~1.5 for strided
    reduce), then combine blocks per segment with small strided reduces.
    """
    BLK = 32 * D  # one folded 32-field block: 2048 elems
    if variant == "strided":
        t3 = t[:].rearrange("b (f d) -> b d f", d=D)
        for si in range(NSEG_G):
            f0, sz = SEG_OFF[si], SEG_SZ[si]
            nc.vector.reduce_sum(
                out=o[:, si * D : (si + 1) * D],
                in_=t3[:, :, f0 : f0 + sz],
                axis=mybir.AxisListType.X,
            )
            nc.scalar.mul(
                out=o[:, si * D : (si + 1) * D],
                in_=o[:, si * D : (si + 1) * D],
                mul=1.0 / sz,
            )
        return

    if variant in ("tree", "tree_gps"):
        # view [b, blk, within]: fold `within` 1024->512->...->64 in place.
        # tree_gps: blocks 6-7 (segment 3) fold on GPSIMD instead of DVE.
        nk = 6 if variant == "tree_gps" else 8
        for width in (1024, 512, 256, 128, 64):
            v = t[:].rearrange("b (k w) -> b k w", w=BLK)
            nc.vector.tensor_add(
                v[:, :nk, :width], v[:, :nk, :width],
                v[:, :nk, width : 2 * width],
            )
            if variant == "tree_gps":
                nc.gpsimd.tensor_add(
                    v[:, 6:, :width], v[:, 6:, :width],
                    v[:, 6:, width : 2 * width],
                )
        if variant == "tree_gps":
            o3 = o[:, 3 * D : 4 * D]
            nc.gpsimd.tensor_add(
                o3, t[:, 6 * BLK : 6 * BLK + D], t[:, 7 * BLK : 7 * BLK + D]
            )
            nc.gpsimd.tensor_scalar_mul(o3, o3, 1.0 / SEG_SZ[3])
        # block sums now at t[:, k*BLK : k*BLK + 64] for k in 0..7
        blocks = t[:].rearrange("b (k w) -> b w k", w=BLK)[:, :D, :]
        seg_blocks = ((0, 1), (1, 3), (3, 6), (6, 8))
        for si, (k0, k1) in enumerate(seg_blocks):
            if variant == "tree_gps" and si == 3:
                continue  # handled on GPSIMD above
            osl = o[:, si * D : (si + 1) * D]
            if k1 - k0 == 1:
                nc.scalar.activation(
                    out=osl,
                    in_=t[:, k0 * BLK : k0 * BLK + D],
                    func=mybir.ActivationFunctionType.Copy,
                    scale=1.0 / SEG_SZ[si],
                )
            else:
                nc.vector.reduce_sum(
                    out=osl, in_=blocks[:, :, k0:k1], axis=mybir.AxisListType.X
                )
                nc.scalar.mul(out=osl, in_=osl, mul=1.0 / SEG_SZ[si])
        return

    assert variant == "hybrid"
    # One contiguous in-place fold level (each 32-field block: fields
    # [0:16) += [16:32)), then one strided XY-reduce per segment over the
    # folded fields of its blocks.
    v = t[:].rearrange("b (k w) -> b k w", w=BLK)
    nc.vector.tensor_add(v[:, :, :1024], v[:, :, :1024], v[:, :, 1024:2048])
    # folded tile view [b, k, f(16), d] -> reduce per segment over (k, f)
    t4 = t[:].rearrange("b (k f d) -> b d k f", k=8, d=D)  # [b, d, k, f16]
    seg_blocks = ((0, 1), (1, 3), (3, 6), (6, 8))
    for si, (k0, k1) in enumerate(seg_blocks):
        osl = o[:, si * D : (si + 1) * D]
        nc.vector.reduce_sum(
            out=osl,
            in_=t4[:, :, k0:k1, :16],
            axis=mybir.AxisListType.XY,
        )
        nc.scalar.mul(out=osl, in_=osl, mul=1.0 / SEG_SZ[si])


@lru_cache(maxsize=16)
def _build(reps: int = 1, variant: str = "tree_gps", chunk_f: int = 256,
           bufs: int = 2, out_eng: str = "scalar"):
    """reps>1 repeats the whole workload back-to-back inside one NEFF —
    used only for timing (marginal per-rep time cancels dispatch+preamble
    overheads)."""
    nc = bacc.Bacc(
        "TRN2", target_bir_lowering=False, debug=False, num_devices=N_CORES
    )
    x = nc.declare_dram_parameter("x", [B_LOC, FIELDS, D], FP32, isOutput=False)
    y = nc.declare_dram_parameter("y", [B_LOC, NSEG, D], FP32, isOutput=True)
    xf = x.rearrange("b f d -> b (f d)")

    with tile.TileContext(nc) as tc:
        with (
            tc.tile_pool(name="inp", bufs=bufs) as inp_pool,
            tc.tile_pool(name="outp", bufs=2) as out_pool,
            tc.tile_pool(name="tmpp", bufs=2) as tmp_pool,
        ):
            for _ in range(reps):
                if chunk_f == GROUP_F:
                    o_all = None
                    if out_eng == "final":
                        o_all = out_pool.tile([B_LOC, NSEG * D], FP32,
                                              tag="oall")
                    for g in range(GROUPS):
                        t = inp_pool.tile(
                            [B_LOC, GROUP_F * D], FP32, tag="in"
                        )
                        nc.sync.dma_start(
                            out=t[:],
                            in_=xf[:, g * GROUP_F * D : (g + 1) * GROUP_F * D],
                        )
                        if out_eng == "final":
                            o = o_all[:, g * NSEG_G * D : (g + 1) * NSEG_G * D]
                            _emit_group(nc, t, o, variant)
                        else:
                            o = out_pool.tile([B_LOC, NSEG_G * D], FP32,
                                              tag="out")
                            _emit_group(nc, t, o[:], variant)
                            dma_eng = {
                                "sync": nc.sync,
                                "gpsimd": nc.gpsimd,
                                "scalar": nc.scalar,
                            }[out_eng]
                            dma_eng.dma_start(
                                out=y[:, g * NSEG_G : (g + 1) * NSEG_G, :],
                                in_=o[:].rearrange("b (s d) -> b s d", d=D),
                            )
                    if out_eng == "final":
                        nc.scalar.dma_start(
                            out=y[:, :, :],
                            in_=o_all[:].rearrange("b (s d) -> b s d", d=D),
                        )
                else:
                    assert chunk_f == GROUP_F // 2 and variant == "strided"
                    HF = chunk_f * D  # 8192
                    for g in range(GROUPS):
                        o = out_pool.tile([B_LOC, NSEG_G * D], FP32, tag="out")
                        for h in range(2):
                            t = inp_pool.tile([B_LOC, HF], FP32, tag="in")
                            nc.sync.dma_start(
                                out=t[:],
                                in_=xf[
                                    :,
                                    (2 * g + h) * HF : (2 * g + h + 1) * HF,
                                ],
                            )
                            t3 = t[:].rearrange("b (f d) -> b d f", d=D)
                            if h == 0:
                                # fields 0:128 = seg0(32), seg1(64), seg2a(32)
                                nc.vector.reduce_sum(
                                    out=o[:, 0:D], in_=t3[:, :, 0:32],
                                    axis=mybir.AxisListType.X,
                                )
                                nc.vector.reduce_sum(
                                    out=o[:, D : 2 * D], in_=t3[:, :, 32:96],
                                    axis=mybir.AxisListType.X,
                                )
                                nc.vector.reduce_sum(
                                    out=o[:, 2 * D : 3 * D],
                                    in_=t3[:, :, 96:128],
                                    axis=mybir.AxisListType.X,
                                )
                            else:
                                # fields 128:256 = seg2b(64), seg3(64)
                                tmp = tmp_pool.tile([B_LOC, D], FP32, tag="t2")
                                nc.vector.reduce_sum(
                                    out=tmp[:], in_=t3[:, :, 0:64],
                                    axis=mybir.AxisListType.X,
                                )
                                nc.vector.tensor_add(
                                    o[:, 2 * D : 3 * D], o[:, 2 * D : 3 * D],
                                    tmp[:],
                                )
                                nc.vector.reduce_sum(
                                    out=o[:, 3 * D : 4 * D],
                                    in_=t3[:, :, 64:128],
                                    axis=mybir.AxisListType.X,
                                )
                        for si in range(NSEG_G):
                            nc.scalar.mul(
                                out=o[:, si * D : (si + 1) * D],
                                in_=o[:, si * D : (si + 1) * D],
                                mul=1.0 / SEG_SZ[si],
                            )
                        dma_eng = nc.sync if out_eng == "sync" else nc.gpsimd
                        dma_eng.dma_start(
                            out=y[:, g * NSEG_G : (g + 1) * NSEG_G, :],
                            in_=o[:].rearrange("b (s d) -> b s d", d=D),
                        )
    nc.finalize()
    return nc


def _sharded_from_nc(nc):
    """Build the 8-way-sharded jitted executable for a finalized Bass module.

    Mirrors bass2jax.run_bass_via_pjrt's multi-core branch (shard_map over a
    'core' mesh; per-device shard == the BIR-declared per-core shape) but
    without output-buffer donation so the same function can be called in a
    timing loop with device-resident inputs.
    """
    import jax
    from jax.experimental.shard_map import shard_map
    from jax.sharding import Mesh, NamedSharding, PartitionSpec

    from concourse import bass2jax, mybir as _mybir

    bass2jax.install_neuronx_cc_hook()

    in_names, out_names, out_avals, zero_outs = [], [], [], []
    partition_name = (
        nc.partition_id_tensor.name if nc.partition_id_tensor else None
    )
    for alloc in nc.m.functions[0].allocations:
        if not isinstance(alloc, _mybir.MemoryLocationSet):
            continue
        name = alloc.memorylocations[0].name
        if alloc.kind == "ExternalInput":
            if name != partition_name:
                in_names.append(name)
        elif alloc.kind == "ExternalOutput":
            shape = tuple(alloc.tensor_shape)
            dtype = _mybir.dt.np(alloc.dtype)
            out_names.append(name)
            out_avals.append(jax.core.ShapedArray(shape, dtype))
            zero_outs.append(np.zeros(shape, dtype))
    n_params = len(in_names)
    all_in_names = list(in_names) + list(out_names)
    if partition_name is not None:
        all_in_names.append(partition_name)

    def _body(*args):
        operands = list(args)
        if partition_name is not None:
            operands.append(bass2jax.partition_id_tensor())
        outs = bass2jax._bass_exec_p.bind(
            *operands,
            out_avals=tuple(out_avals),
            in_names=tuple(all_in_names),
            out_names=tuple(out_names),
            lowering_input_output_aliases=(),
            sim_require_finite=True,
            sim_require_nnan=True,
            nc=nc,
        )
        return tuple(outs)

    devices = jax.devices()[:N_CORES]
    mesh = Mesh(np.asarray(devices), ("core",))
    n_outs = len(out_names)
    in_specs = (PartitionSpec("core"),) * (n_params + n_outs)
    out_specs = (PartitionSpec("core"),) * n_outs
    sharded = jax.jit(
        shard_map(
            _body, mesh=mesh, in_specs=in_specs, out_specs=out_specs,
            check_rep=False,
        ),
        keep_unused=True,
    )
    in_sharding = NamedSharding(mesh, PartitionSpec("core"))
    return sharded, zero_outs, in_sharding


@lru_cache(maxsize=4)
def _compiled(reps: int = 1):
    return _sharded_from_nc(_build(reps))


def _put_inputs(emb_vector: np.ndarray, reps: int = 1):
    import jax

    sharded, zero_outs, in_sharding = _compiled(reps)
    x = np.ascontiguousarray(emb_vector, dtype=np.float32)
    dx = jax.device_put(x, in_sharding)
    dzeros = [
        jax.device_put(
            np.zeros((N_CORES * z.shape[0], *z.shape[1:]), z.dtype), in_sharding
        )
        for z in zero_outs
    ]
    return sharded, dx, dzeros


def kernel(emb_vector: np.ndarray) -> np.ndarray:
    sharded, dx, dzeros = _put_inputs(emb_vector)
    (out,) = sharded(dx, *dzeros)
    return np.asarray(out)


def bench(emb_vector: np.ndarray, iters: int = 30, warmup: int = 5,
          reps: int = 1):
    """Steady-state per-call wall time of the sharded executable, ns."""
    import time

    sharded, dx, dzeros = _put_inputs(emb_vector, reps)
    for _ in range(warmup):
        (out,) = sharded(dx, *dzeros)
    out.block_until_ready()
    t0 = time.perf_counter()
    for _ in range(iters):
        (out,) = sharded(dx, *dzeros)
    out.block_until_ready()
    t1 = time.perf_counter()
    return (t1 - t0) / iters * 1e9, np.asarray(out)


def measure_exec_ns(emb_vector: np.ndarray, lo: int = 4, hi: int = 12,
                    iters: int = 20, n_pairs: int = 7):
    """Marginal per-execution HW time via in-NEFF workload repetition:
    (t(hi reps) - t(lo reps)) / (hi - lo) cancels per-dispatch client/RPC
    overhead and NEFF preamble/postamble. hi/lo timing loops are
    interleaved (median of per-pair diffs) so device-load drift cancels."""
    import time

    sharded_hi, dx, dz_hi = _put_inputs(emb_vector, hi)
    sharded_lo, _, dz_lo = _put_inputs(emb_vector, lo)
    for _ in range(4):
        (out,) = sharded_hi(dx, *dz_hi)
        (out_lo,) = sharded_lo(dx, *dz_lo)
    out.block_until_ready()
    out_lo.block_until_ready()
    diffs = []
    for _ in range(n_pairs):
        t0 = time.perf_counter()
        for _ in range(iters):
            (out,) = sharded_hi(dx, *dz_hi)
        out.block_until_ready()
        t1 = time.perf_counter()
        for _ in range(iters):
            (out_lo,) = sharded_lo(dx, *dz_lo)
        out_lo.block_until_ready()
        t2 = time.perf_counter()
        diffs.append(((t1 - t0) - (t2 - t1)) / iters * 1e9)
    med = sorted(diffs)[len(diffs) // 2]
    return med / (hi - lo), np.asarray(out)
